# revision 14
# baseline (speedup 1.0000x reference)
"""Last-query sparse attention on 8 TRN2 NeuronCores.

Reference computation (per sample b):
    prev  = x[b, :-1, :]                 # [T-1, D]
    final = x[b, -1, :]                  # [D]
    s     = prev @ final                 # [T-1]
    w     = softmax(s)
    att   = w @ prev                     # [D]
    out   = concat(final, att)           # [2D]

Sharding: batch (B=64) split 8 ways -> 8 samples per core, no collectives.

Design notes (v4):
- DMA-bound kernel: 33.55MB f32 HBM read + 16.78MB fp16 SBUF write per
  core. SDMA engine 15 is ~15% slower than engines 0-14 (SWDGE ring
  contention), so rows are assigned non-uniformly: partitions served by
  engine 15 ({92..95, 124..127}) hold 28 rows of x[b], partitions 0..31
  hold 33, the rest 32 (total 4096). Unused (p, i) slots are zeroed once
  (zero products -> score 0 -> weight ~ e^-55 ~ 0 since gmax ~ 55).
- The GpSimd queue carries ONLY the load DMAs. The tile framework paces
  DMA issue through 8 completion-semaphore lanes, so anything else on
  that queue (e.g. a partition_all_reduce) head-blocks descriptor
  generation and starves the SDMA engines mid-kernel.
- Engine queues are strict FIFO, so the loop is software-pipelined: the
  Vector queue per sample runs [pass-1(b), rowmax(b), epilogue(b-1),
  gmax-reduce(b)] -- every op is (nearly) data-ready when the queue head
  reaches it.
- Softmax without GpSimd: row max (DVE, fp16) -> one-column matmul
  against an identity transposes it to partition 0 (PE) -> row max
  (DVE) -> negated-ones matmul broadcasts -gmax to all partitions (PE)
  -> ACT copies it out of PSUM and applies exp. The denominator is a
  ones-vector matmul over the exp weights (PE) + row reduce (DVE).
- Pass 2: 33 matmuls (lhsT = weight column [128,1], rhs = fp16 X block
  [128,256]) alternating between two PSUM banks (avoids same-bank
  accumulate stalls); att = bankA + bankB (one ACT PSUM->SBUF copy),
  scaled by 1/Z, stored per sample. The PE queue is a single warm
  stream: [... pass2(b-1), Z(b-1), transpose(b), bcast(b), pass2(b) ...]
- The last sample loads in 4 chunks so its pass-1 rides the DMA tail.
"""

import sys

sys.path.insert(0, "/opt/trn_rl_repo")

from contextlib import ExitStack

import numpy as np

import concourse.tile as tile
from concourse import bacc, mybir
from concourse.bass_utils import run_bass_kernel_spmd

N_CORES = 8
B = 64
T = 4096
D = 256
BPC = B // N_CORES  # samples per core
P = 128
NBLK = 33  # padded block count; t rows are distributed non-uniformly
F32 = mybir.dt.float32
FP16 = mybir.dt.float16

# (p0, p1, rows, row_offset): partition range [p0,p1) holds `rows`
# contiguous rows of x[b] starting at row_offset + (p-p0)*rows.
RANGES = [
    (0, 32, 33, 0),
    (32, 92, 32, 1056),
    (92, 96, 28, 2976),
    (96, 124, 32, 3088),
    (124, 128, 28, 3984),
]
MASK_COL = 27  # self-score: row 4095 lives at partition 127, block 27
CHUNKS = [(0, 17), (17, 33)]  # pass-1 chunks
CHUNKS_LAST = [(0, 9), (9, 17), (17, 25), (25, 33)]

_NC_CACHE = None


def _build():
    nc = bacc.Bacc(
        trn_type="TRN2",
        target_bir_lowering=False,
        debug=False,
        num_devices=N_CORES,
    )
    x_ext = nc.declare_dram_parameter("x", [BPC, T, D], F32, isOutput=False)
    ident_ext = nc.declare_dram_parameter("cst_ident", [P, P], FP16, isOutput=False)
    ones_ext = nc.declare_dram_parameter("cst_ones", [P, 1], FP16, isOutput=False)
    nones_ext = nc.declare_dram_parameter("cst_negones", [1, P], FP16, isOutput=False)
    mask_ext = nc.declare_dram_parameter("cst_mask", [P, 1], F32, isOutput=False)
    zero_ext = nc.declare_dram_parameter("cst_zeros", [4, 5, D], FP16, isOutput=False)
    out_ext = nc.declare_dram_parameter("out", [BPC, 2 * D], F32, isOutput=True)
    xap = x_ext.ap()
    oap = out_ext.ap()

    with ExitStack() as ctx:
        tc = ctx.enter_context(tile.TileContext(nc))
        xpool = ctx.enter_context(tc.tile_pool(name="xp", bufs=8))
        fpool = ctx.enter_context(tc.tile_pool(name="fp", bufs=8))
        fhpool = ctx.enter_context(tc.tile_pool(name="fhp", bufs=8))
        scr = ctx.enter_context(tc.tile_pool(name="scr", bufs=3))
        spool = ctx.enter_context(tc.tile_pool(name="sp", bufs=3))
        pwpool = ctx.enter_context(tc.tile_pool(name="pw", bufs=2))
        stat = ctx.enter_context(tc.tile_pool(name="stat", bufs=2))
        cpool = ctx.enter_context(tc.tile_pool(name="const", bufs=1))
        psa = ctx.enter_context(tc.tile_pool(name="psa", bufs=2, space="PSUM"))
        psb = ctx.enter_context(tc.tile_pool(name="psb", bufs=2, space="PSUM"))
        psx = ctx.enter_context(tc.tile_pool(name="psx", bufs=2, space="PSUM"))
        psn = ctx.enter_context(tc.tile_pool(name="psn", bufs=2, space="PSUM"))

        ident16 = cpool.tile([P, P], FP16)
        nc.sync.dma_start(ident16[:], ident_ext.ap())
        ones16 = cpool.tile([P, 1], FP16)
        nc.sync.dma_start(ones16[:], ones_ext.ap())
        negones16 = cpool.tile([1, P], FP16)
        nc.sync.dma_start(negones16[:], nones_ext.ap())
        maskbias = cpool.tile([P, 1], F32)
        nc.sync.dma_start(maskbias[:], mask_ext.ap())

        xtiles = [
            xpool.tile([P, NBLK, D], FP16, tag="xh", name=f"xh{b}") for b in range(BPC)
        ]

        # ---- pad init + all load issues (GpSimd queue: loads only) ----
        fhtiles = []
        for b in range(BPC):
            xt = xtiles[b]
            nc.vector.memset(xt[32:64, 32:NBLK, :], 0.0)
            nc.vector.memset(xt[64:96, 32:NBLK, :], 0.0)
            nc.vector.memset(xt[96:124, 32:NBLK, :], 0.0)
            nc.sync.dma_start(xt[92:96, 28:NBLK, :], zero_ext.ap())
            nc.sync.dma_start(xt[124:128, 28:NBLK, :], zero_ext.ap())

            csplits = CHUNKS_LAST if b == BPC - 1 else [(0, NBLK)]
            for c0, c1 in csplits:
                for p0, p1, rows, off in RANGES:
                    r0, r1 = min(c0, rows), min(c1, rows)
                    if r1 <= r0:
                        continue
                    src = xap[b, off : off + (p1 - p0) * rows].rearrange(
                        "(p i) d -> p i d", p=p1 - p0
                    )[:, r0:r1, :]
                    nc.gpsimd.dma_start(xt[p0:p1, r0:r1, :], src)
            F = fpool.tile([P, D], F32, tag="f", name=f"f{b}")
            nc.sync.dma_start(F[:], xap[b, T - 1].partition_broadcast(P))
            nc.sync.dma_start(oap[b : b + 1, 0:D], F[0:1, :])
            Fh = fhpool.tile([P, D], FP16, tag="fh", name=f"fh{b}")
            nc.scalar.copy(Fh[:], F[:])
            fhtiles.append(Fh)

        # ---- software-pipelined compute ----
        pend = {}  # b -> (pA, pB, pZ)

        def epilogue(b):
            """Denominator + normalize + store for sample b. Issued after
            sample b+1's pass-1 so the Vector queue never stalls on PE."""
            pA, pB, pZ = pend.pop(b)
            att_b = stat.tile([1, D], F32, tag="ab", name=f"ab{b}")
            nc.scalar.copy(att_b[:], pB[:])
            z = stat.tile([1, 1], F32, tag="z", name=f"z{b}")
            nc.vector.reduce_sum(z[:], pZ[:, 0:NBLK], axis=mybir.AxisListType.X)
            rz = stat.tile([1, 1], F32, tag="rz", name=f"rz{b}")
            nc.vector.reciprocal(rz[:], z[:])
            att_u = stat.tile([1, D], F32, tag="au", name=f"au{b}")
            nc.vector.tensor_add(att_u[:], pA[:], att_b[:])
            att_n = stat.tile([1, D], F32, tag="an", name=f"an{b}")
            nc.vector.tensor_mul(att_n[:], att_u[:], rz[:].broadcast_to((1, D)))
            nc.sync.dma_start(oap[b : b + 1, D : 2 * D], att_n[:])

        for b in range(BPC):
            xt = xtiles[b]
            Fh = fhtiles[b]

            # DVE: pass-1 -> scores
            S = spool.tile([P, NBLK], F32, tag="s", name=f"s{b}")
            chunks = CHUNKS_LAST if b == BPC - 1 else CHUNKS
            for c0, c1 in chunks:
                cn = c1 - c0
                prod = scr.tile([P, 17, D], FP16, tag="prod", name=f"pr{b}_{c0}")
                nc.vector.tensor_mul(
                    prod[:, 0:cn, :],
                    xt[:, c0:c1, :],
                    Fh[:].unsqueeze(1).broadcast_to((P, cn, D)),
                )
                l1 = scr.tile([P, 17, D // 2], FP16, tag="l1", name=f"l1_{b}_{c0}")
                nc.vector.tensor_add(
                    l1[:, 0:cn, :],
                    prod[:, 0:cn, 0 : D // 2],
                    prod[:, 0:cn, D // 2 : D],
                )
                l2 = scr.tile([P, 17, D // 4], FP16, tag="l2", name=f"l2_{b}_{c0}")
                nc.vector.tensor_add(
                    l2[:, 0:cn, :],
                    l1[:, 0:cn, 0 : D // 4],
                    l1[:, 0:cn, D // 4 : D // 2],
                )
                nc.vector.reduce_sum(
                    S[:, c0:c1], l2[:, 0:cn, :], axis=mybir.AxisListType.X
                )
            nc.vector.tensor_add(
                S[:, MASK_COL : MASK_COL + 1],
                S[:, MASK_COL : MASK_COL + 1],
                maskbias[:],
            )
            rowmax16 = stat.tile([P, 1], FP16, tag="rm", name=f"rm{b}")
            nc.vector.reduce_max(rowmax16[:], S[:], axis=mybir.AxisListType.X)

            # PE: transpose the row maxes to partition 0 (queued right
            # after pass-2(b-1), so it's data-ready when the PE gets here)
            psT = psx.tile([1, P], F32, tag="aux", name=f"pt{b}")
            nc.tensor.matmul(
                psT[:], lhsT=rowmax16[:], rhs=ident16[:], start=True, stop=True
            )

            # interleave sample b-1's epilogue here: keeps the Vector queue
            # busy while pass-2(b-1) finishes on the PE
            if b > 0:
                epilogue(b - 1)

            # DVE: global max; PE: broadcast -gmax; ACT: exp
            gmax16 = stat.tile([1, 1], FP16, tag="gm", name=f"gm{b}")
            nc.vector.reduce_max(gmax16[:], psT[:], axis=mybir.AxisListType.X)
            psN = psn.tile([P, 1], F32, tag="ng", name=f"ng{b}")
            nc.tensor.matmul(
                psN[:], lhsT=negones16[:], rhs=gmax16[:], start=True, stop=True
            )
            negmax = stat.tile([P, 1], F32, tag="nm", name=f"nm{b}")
            nc.scalar.copy(negmax[:], psN[:])
            Pw = pwpool.tile([P, NBLK], FP16, tag="pw", name=f"pw{b}")
            nc.scalar.activation(
                Pw[:],
                S[:],
                mybir.ActivationFunctionType.Exp,
                bias=negmax[:],
                scale=1.0,
            )

            # PE: pass-2 stream, alternating PSUM banks
            pA = psa.tile([1, D], F32, tag="pa", name=f"pa{b}")
            pB = psb.tile([1, D], F32, tag="pb", name=f"pb{b}")
            for i in range(NBLK):
                ps = pA if i % 2 == 0 else pB
                nc.tensor.matmul(
                    ps[:],
                    lhsT=Pw[:, i : i + 1],
                    rhs=xt[:, i, :],
                    start=(i < 2),
                    stop=(i >= NBLK - 2),
                )
            pZ = psx.tile([1, P], F32, tag="aux", name=f"pz{b}")
            nc.tensor.matmul(
                pZ[:, 0:NBLK], lhsT=ones16[:], rhs=Pw[:], start=True, stop=True
            )
            pend[b] = (pA, pB, pZ)

        epilogue(BPC - 1)

    nc.compile()
    return nc


def _consts():
    return {
        "cst_ident": np.eye(P, dtype=np.float16),
        "cst_ones": np.ones((P, 1), dtype=np.float16),
        "cst_negones": np.full((1, P), -1.0, dtype=np.float16),
        "cst_mask": np.concatenate(
            [np.zeros((P - 1, 1), np.float32), np.full((1, 1), -1.0e30, np.float32)]
        ),
        "cst_zeros": np.zeros((4, 5, D), dtype=np.float16),
    }


def _run(x, trace=False):
    global _NC_CACHE
    x = np.ascontiguousarray(np.asarray(x, dtype=np.float32))
    assert x.shape == (B, T, D), x.shape
    if _NC_CACHE is None:
        _NC_CACHE = _build()
    cst = _consts()
    in_maps = [{"x": x[c * BPC : (c + 1) * BPC], **cst} for c in range(N_CORES)]
    res = run_bass_kernel_spmd(
        _NC_CACHE, in_maps, core_ids=list(range(N_CORES)), trace=trace
    )
    out = np.concatenate([res.results[c]["out"] for c in range(N_CORES)], axis=0)
    return out.astype(np.float32), res


def kernel(x):
    out, _ = _run(x, trace=False)
    return out


# revision 16
# speedup vs baseline: 1.1221x; 1.1221x over previous
"""Last-query sparse attention on 8 TRN2 NeuronCores.

Reference computation (per sample b):
    prev  = x[b, :-1, :]                 # [T-1, D]
    final = x[b, -1, :]                  # [D]
    s     = prev @ final                 # [T-1]
    w     = softmax(s)
    att   = w @ prev                     # [D]
    out   = concat(final, att)           # [2D]

Sharding: batch (B=64) split 8 ways -> 8 samples per core, no collectives.

Design notes (v4):
- DMA-bound kernel: 33.55MB f32 HBM read + 16.78MB fp16 SBUF write per
  core. SDMA engine 15 is ~15% slower than engines 0-14 (SWDGE ring
  contention), so rows are assigned non-uniformly: partitions served by
  engine 15 ({92..95, 124..127}) hold 28 rows of x[b], partitions 0..31
  hold 33, the rest 32 (total 4096). Unused (p, i) slots are zeroed once
  (zero products -> score 0 -> weight ~ e^-55 ~ 0 since gmax ~ 55).
- The GpSimd queue carries ONLY the load DMAs. The tile framework paces
  DMA issue through 8 completion-semaphore lanes, so anything else on
  that queue (e.g. a partition_all_reduce) head-blocks descriptor
  generation and starves the SDMA engines mid-kernel.
- Engine queues are strict FIFO, so the loop is software-pipelined: the
  Vector queue per sample runs [pass-1(b), rowmax(b), epilogue(b-1),
  gmax-reduce(b)] -- every op is (nearly) data-ready when the queue head
  reaches it.
- Softmax without GpSimd: row max (DVE, fp16) -> one-column matmul
  against an identity transposes it to partition 0 (PE) -> row max
  (DVE) -> negated-ones matmul broadcasts -gmax to all partitions (PE)
  -> ACT copies it out of PSUM and applies exp. The denominator is a
  ones-vector matmul over the exp weights (PE) + row reduce (DVE).
- Pass 2: 33 matmuls (lhsT = weight column [128,1], rhs = fp16 X block
  [128,256]) alternating between two PSUM banks (avoids same-bank
  accumulate stalls); att = bankA + bankB (one ACT PSUM->SBUF copy),
  scaled by 1/Z, stored per sample. The PE queue is a single warm
  stream: [... pass2(b-1), Z(b-1), transpose(b), bcast(b), pass2(b) ...]
- The last sample loads in 4 chunks so its pass-1 rides the DMA tail.
"""

import sys

sys.path.insert(0, "/opt/trn_rl_repo")

from contextlib import ExitStack

import numpy as np

import concourse.tile as tile
from concourse import bacc, mybir
from concourse.bass_utils import run_bass_kernel_spmd

N_CORES = 8
B = 64
T = 4096
D = 256
BPC = B // N_CORES  # samples per core
P = 128
NBLK = 33  # padded block count; t rows are distributed non-uniformly
F32 = mybir.dt.float32
FP16 = mybir.dt.float16

# (p0, p1, rows, row_offset): partition range [p0,p1) holds `rows`
# contiguous rows of x[b] starting at row_offset + (p-p0)*rows.
RANGES = [
    (0, 32, 33, 0),
    (32, 92, 32, 1056),
    (92, 96, 28, 2976),
    (96, 124, 32, 3088),
    (124, 128, 28, 3984),
]
MASK_COL = 27  # self-score: row 4095 lives at partition 127, block 27
CHUNKS = [(0, 17), (17, 33)]  # pass-1 chunks
CHUNKS_LAST = [(0, 9), (9, 17), (17, 25), (25, 33)]

_NC_CACHE = None


def _build():
    nc = bacc.Bacc(
        trn_type="TRN2",
        target_bir_lowering=False,
        debug=False,
        num_devices=N_CORES,
    )
    x_ext = nc.declare_dram_parameter("x", [BPC, T, D], F32, isOutput=False)
    ident_ext = nc.declare_dram_parameter("cst_ident", [P, P], FP16, isOutput=False)
    ones_ext = nc.declare_dram_parameter("cst_ones", [P, 1], FP16, isOutput=False)
    nones_ext = nc.declare_dram_parameter("cst_negones", [1, P], FP16, isOutput=False)
    mask_ext = nc.declare_dram_parameter("cst_mask", [P, 1], F32, isOutput=False)
    zero_ext = nc.declare_dram_parameter("cst_zeros", [4, 5, D], FP16, isOutput=False)
    out_ext = nc.declare_dram_parameter("out", [BPC, 2 * D], F32, isOutput=True)
    xap = x_ext.ap()
    oap = out_ext.ap()

    with ExitStack() as ctx:
        tc = ctx.enter_context(tile.TileContext(nc))
        xpool = ctx.enter_context(tc.tile_pool(name="xp", bufs=8))
        fpool = ctx.enter_context(tc.tile_pool(name="fp", bufs=8))
        fhpool = ctx.enter_context(tc.tile_pool(name="fhp", bufs=8))
        scr = ctx.enter_context(tc.tile_pool(name="scr", bufs=3))
        spool = ctx.enter_context(tc.tile_pool(name="sp", bufs=3))
        pwpool = ctx.enter_context(tc.tile_pool(name="pw", bufs=2))
        stat = ctx.enter_context(tc.tile_pool(name="stat", bufs=2))
        cpool = ctx.enter_context(tc.tile_pool(name="const", bufs=1))
        psa = ctx.enter_context(tc.tile_pool(name="psa", bufs=2, space="PSUM"))
        psb = ctx.enter_context(tc.tile_pool(name="psb", bufs=2, space="PSUM"))
        psx = ctx.enter_context(tc.tile_pool(name="psx", bufs=2, space="PSUM"))
        psn = ctx.enter_context(tc.tile_pool(name="psn", bufs=2, space="PSUM"))

        ident16 = cpool.tile([P, P], FP16)
        nc.sync.dma_start(ident16[:], ident_ext.ap())
        ones16 = cpool.tile([P, 1], FP16)
        nc.sync.dma_start(ones16[:], ones_ext.ap())
        negones16 = cpool.tile([1, P], FP16)
        nc.sync.dma_start(negones16[:], nones_ext.ap())
        maskbias = cpool.tile([P, 1], F32)
        nc.sync.dma_start(maskbias[:], mask_ext.ap())

        xtiles = [
            xpool.tile([P, NBLK, D], FP16, tag="xh", name=f"xh{b}") for b in range(BPC)
        ]

        # ---- pad init + all big-load issues (GpSimd queue: loads only) ----
        for b in range(BPC):
            xt = xtiles[b]
            nc.vector.memset(xt[32:64, 32:NBLK, :], 0.0)
            nc.vector.memset(xt[64:96, 32:NBLK, :], 0.0)
            nc.vector.memset(xt[96:124, 32:NBLK, :], 0.0)

            csplits = CHUNKS_LAST if b == BPC - 1 else [(0, NBLK)]
            for c0, c1 in csplits:
                for p0, p1, rows, off in RANGES:
                    r0, r1 = min(c0, rows), min(c1, rows)
                    if r1 <= r0:
                        continue
                    src = xap[b, off : off + (p1 - p0) * rows].rearrange(
                        "(p i) d -> p i d", p=p1 - p0
                    )[:, r0:r1, :]
                    nc.gpsimd.dma_start(xt[p0:p1, r0:r1, :], src)

        # F-row machinery is staggered per sample (3 ahead of use): under
        # the SWDGE flood an HWDGE broadcast takes 10-30us to complete, and
        # the ACT queue is FIFO -- queueing all 8 casts upfront would gate
        # sample 0's softmax on sample 7's F load.
        fhtiles = {}

        def fstuff(b):
            xt = xtiles[b]
            nc.sync.dma_start(xt[92:96, 28:NBLK, :], zero_ext.ap())
            nc.sync.dma_start(xt[124:128, 28:NBLK, :], zero_ext.ap())
            F = fpool.tile([P, D], F32, tag="f", name=f"f{b}")
            nc.sync.dma_start(F[:], xap[b, T - 1].partition_broadcast(P))
            nc.sync.dma_start(oap[b : b + 1, 0:D], F[0:1, :])
            Fh = fhpool.tile([P, D], FP16, tag="fh", name=f"fh{b}")
            nc.scalar.copy(Fh[:], F[:])
            fhtiles[b] = Fh

        # ---- software-pipelined compute ----
        pend = {}  # b -> (pA, pB, pZ)

        def epilogue(b):
            """Denominator + normalize + store for sample b. Issued after
            sample b+1's pass-1 so the Vector queue never stalls on PE."""
            pA, pB, pZ = pend.pop(b)
            att_b = stat.tile([1, D], F32, tag="ab", name=f"ab{b}")
            nc.scalar.copy(att_b[:], pB[:])
            z = stat.tile([1, 1], F32, tag="z", name=f"z{b}")
            nc.vector.reduce_sum(z[:], pZ[:, 0:NBLK], axis=mybir.AxisListType.X)
            rz = stat.tile([1, 1], F32, tag="rz", name=f"rz{b}")
            nc.vector.reciprocal(rz[:], z[:])
            att_u = stat.tile([1, D], F32, tag="au", name=f"au{b}")
            nc.vector.tensor_add(att_u[:], pA[:], att_b[:])
            att_n = stat.tile([1, D], F32, tag="an", name=f"an{b}")
            nc.vector.tensor_mul(att_n[:], att_u[:], rz[:].broadcast_to((1, D)))
            nc.sync.dma_start(oap[b : b + 1, D : 2 * D], att_n[:])

        for b in range(3):
            fstuff(b)
        for b in range(BPC):
            if b + 3 < BPC:
                fstuff(b + 3)
            xt = xtiles[b]
            Fh = fhtiles[b]

            # DVE: pass-1 -> scores
            S = spool.tile([P, NBLK], F32, tag="s", name=f"s{b}")
            chunks = CHUNKS_LAST if b == BPC - 1 else CHUNKS
            for c0, c1 in chunks:
                cn = c1 - c0
                prod = scr.tile([P, 17, D], FP16, tag="prod", name=f"pr{b}_{c0}")
                nc.vector.tensor_mul(
                    prod[:, 0:cn, :],
                    xt[:, c0:c1, :],
                    Fh[:].unsqueeze(1).broadcast_to((P, cn, D)),
                )
                l1 = scr.tile([P, 17, D // 2], FP16, tag="l1", name=f"l1_{b}_{c0}")
                nc.vector.tensor_add(
                    l1[:, 0:cn, :],
                    prod[:, 0:cn, 0 : D // 2],
                    prod[:, 0:cn, D // 2 : D],
                )
                l2 = scr.tile([P, 17, D // 4], FP16, tag="l2", name=f"l2_{b}_{c0}")
                nc.vector.tensor_add(
                    l2[:, 0:cn, :],
                    l1[:, 0:cn, 0 : D // 4],
                    l1[:, 0:cn, D // 4 : D // 2],
                )
                nc.vector.reduce_sum(
                    S[:, c0:c1], l2[:, 0:cn, :], axis=mybir.AxisListType.X
                )
            nc.vector.tensor_add(
                S[:, MASK_COL : MASK_COL + 1],
                S[:, MASK_COL : MASK_COL + 1],
                maskbias[:],
            )
            rowmax16 = stat.tile([P, 1], FP16, tag="rm", name=f"rm{b}")
            nc.vector.reduce_max(rowmax16[:], S[:], axis=mybir.AxisListType.X)

            # PE: transpose the row maxes to partition 0 (queued right
            # after pass-2(b-1), so it's data-ready when the PE gets here)
            psT = psx.tile([1, P], F32, tag="aux", name=f"pt{b}")
            nc.tensor.matmul(
                psT[:], lhsT=rowmax16[:], rhs=ident16[:], start=True, stop=True
            )

            # interleave sample b-1's epilogue here: keeps the Vector queue
            # busy while pass-2(b-1) finishes on the PE
            if b > 0:
                epilogue(b - 1)

            # DVE: global max; PE: broadcast -gmax; ACT: exp
            gmax16 = stat.tile([1, 1], FP16, tag="gm", name=f"gm{b}")
            nc.vector.reduce_max(gmax16[:], psT[:], axis=mybir.AxisListType.X)
            psN = psn.tile([P, 1], F32, tag="ng", name=f"ng{b}")
            nc.tensor.matmul(
                psN[:], lhsT=negones16[:], rhs=gmax16[:], start=True, stop=True
            )
            negmax = stat.tile([P, 1], F32, tag="nm", name=f"nm{b}")
            nc.scalar.copy(negmax[:], psN[:])
            Pw = pwpool.tile([P, NBLK], FP16, tag="pw", name=f"pw{b}")
            nc.scalar.activation(
                Pw[:],
                S[:],
                mybir.ActivationFunctionType.Exp,
                bias=negmax[:],
                scale=1.0,
            )

            # PE: pass-2 stream, alternating PSUM banks
            pA = psa.tile([1, D], F32, tag="pa", name=f"pa{b}")
            pB = psb.tile([1, D], F32, tag="pb", name=f"pb{b}")
            for i in range(NBLK):
                ps = pA if i % 2 == 0 else pB
                nc.tensor.matmul(
                    ps[:],
                    lhsT=Pw[:, i : i + 1],
                    rhs=xt[:, i, :],
                    start=(i < 2),
                    stop=(i >= NBLK - 2),
                )
            pZ = psx.tile([1, P], F32, tag="aux", name=f"pz{b}")
            nc.tensor.matmul(
                pZ[:, 0:NBLK], lhsT=ones16[:], rhs=Pw[:], start=True, stop=True
            )
            pend[b] = (pA, pB, pZ)

        epilogue(BPC - 1)

    nc.compile()
    return nc


def _consts():
    return {
        "cst_ident": np.eye(P, dtype=np.float16),
        "cst_ones": np.ones((P, 1), dtype=np.float16),
        "cst_negones": np.full((1, P), -1.0, dtype=np.float16),
        "cst_mask": np.concatenate(
            [np.zeros((P - 1, 1), np.float32), np.full((1, 1), -1.0e30, np.float32)]
        ),
        "cst_zeros": np.zeros((4, 5, D), dtype=np.float16),
    }


def _run(x, trace=False):
    global _NC_CACHE
    x = np.ascontiguousarray(np.asarray(x, dtype=np.float32))
    assert x.shape == (B, T, D), x.shape
    if _NC_CACHE is None:
        _NC_CACHE = _build()
    cst = _consts()
    in_maps = [{"x": x[c * BPC : (c + 1) * BPC], **cst} for c in range(N_CORES)]
    res = run_bass_kernel_spmd(
        _NC_CACHE, in_maps, core_ids=list(range(N_CORES)), trace=trace
    )
    out = np.concatenate([res.results[c]["out"] for c in range(N_CORES)], axis=0)
    return out.astype(np.float32), res


def kernel(x):
    out, _ = _run(x, trace=False)
    return out
